# revision 52
# baseline (speedup 1.0000x reference)
"""Causal single-head attention on 8 trn2 NeuronCores (Bass/Tile).

Problem: X [4, 2048, 2048] f32, W_Q/W_K/W_V [2048, 256] f32.
  Z = softmax(mask((X@W_Q)(X@W_K)^T / sqrt(256))) @ (X@W_V)

Sharding: 8 cores = 4 batches x 2 query-stripes. Core (b, s) handles the
queries of batch b at token positions == s (mod 2) -- striping balances the
causal attention work exactly across the two cores of a batch, and makes the
per-core mask structure uniform (the only cross-core difference is whether
the remote stripe's same-index key is visible, which is folded into a tiny
per-core [128,128] additive triangle-mask input).

On-chip layout: the host pre-transposes X to feature-major XT [d_model, rows]
(bf16, partition-major so every DMA is contiguous per partition), so every
projection is a natural PE matmul (contraction on the partition axis).
Attention scores are computed transposed, S^T = K Q^T in [keys, queries]
tiles; exp runs on ScalarE straight out of PSUM (no max subtraction -- score
scale here is ~N(0, 1.8), exp stays well inside f32 range); the softmax
denominator falls out of the P^T @ V_aug matmul via a ones-column appended to
V. Diagonal-band tiles only compute the visible query range, and only the
single triangular 128x128 block gets an additive mask (host-provided input).
All matmuls are bf16; accumulation is f32 in PSUM; final normalize is f32.

Each core projects K/V only for its own stripe; the other stripe's K/V
arrives via two pairwise 2-rank AllGathers (K^T right after the K
projection, V after the V projection).  ncfw cannot move collective data
before its ~45us init wall and serializes same-chip collectives, so the
exchange lands V at ~60-80us; everything V-independent is scheduled before
that point and the PE runs gap-free from the projections to the last
attention matmul.  The bounce DMAs ride the act HWDGE queue (never stuck
behind the input stream on the sync queue); each core then pulls exactly
the pair peer's gather slot into SBUF with conditional DMAs (a per-core
0/1 input register picks the slot; the non-peer DMA is skipped entirely
but still bumps its semaphores for Tile's bookkeeping).

Measured (2026-08-08): 93.3-104.3us depending on collective timing draws.
Closed dead ends (do not retry on this runtime): merged >1MB collective
(RDH crash), remote_dma (unsupported, wedges terminal), AllToAll (2-rank
unsupported), 8-rank V gather (no stream parallelism, 138us), fp8 K
exchange (rel err 2.8e-2 > 2e-2 gate), warmup collective (adds queue
delay), column-split XT loads (halve DMA efficiency).

kernel() takes the FULL inputs and returns the FULL output.
"""

from contextlib import ExitStack

import numpy as np
import ml_dtypes

import concourse.bass as bass
import concourse.tile as tile
from concourse import bacc, mybir
from concourse.bass_utils import run_bass_kernel_spmd

BF16 = mybir.dt.bfloat16
F32 = mybir.dt.float32

B, L, D, DK, DV = 4, 2048, 2048, 256, 256
LQ = L // 2          # queries per core (one stripe)
NT = D // 128        # 16 d_model tiles
KSEG = LQ // 128     # 8 key tiles per segment
CHUNK = 512          # query free-dim chunk for the scores matmul
NCHUNK = LQ // CHUNK
Q4 = CHUNK // 128     # query subtiles per chunk
SCALE = 1.0 / float(np.sqrt(DK))
MASK = -1e9

MODE = "coll2"  # "coll2" | "rdma" (remote-dma exchange; fails on this runtime) | "dup"


def build_kernel(mode: str):
    nc = bacc.Bacc("TRN2", target_bir_lowering=False, debug=False, num_devices=8)

    xcols = LQ if mode == "coll2" else L

    xt_ext = nc.declare_dram_parameter("XT", [128, NT, xcols], BF16, isOutput=False)
    wq_ext = nc.declare_dram_parameter("WQ", [128, NT, DK], BF16, isOutput=False)
    wk_ext = nc.declare_dram_parameter("WK", [128, NT, DK], BF16, isOutput=False)
    wv_ext = nc.declare_dram_parameter("WV", [128, NT, DV], BF16, isOutput=False)
    # trimask[:, seg, :]: additive f32 [128 keys, 128 queries] triangle mask
    # for the diagonal block of the diagonal-band score tiles.
    trimask_ext = nc.declare_dram_parameter("TRIMASK", [128, 2, 128], F32, isOutput=False)
    # conds[0, i] == 1 iff the pair peer's AllGather slot is i (i.e. 1-s == i)
    conds_ext = nc.declare_dram_parameter("CONDS", [1, 2], mybir.dt.uint32, isOutput=False)
    out_ext = nc.declare_dram_parameter("OUT", [LQ, DV], F32, isOutput=True)

    # DRAM bounce buffers for the pairwise K/V exchange (bf16), coll2 only.
    # rdma mode pushes SBUF->SBUF directly to the pair peer (core^1) via
    # remote_dma_broadcast with a relative dest (0, 1) -- the SWDGE Q7 XORs
    # the delta with its own (routing_id, tpb_idx), so the same program
    # addresses each core's neighbor with a compile-time constant, bypassing
    # the ncfw collectives path (and its ~45us init wall) entirely.
    vcols = KSEG * (DV + 1)  # 2056
    if mode == "coll2":
        kt_bounce = nc.dram_tensor("kt_bounce", [128, 2 * LQ], BF16)
        kt_gat = nc.dram_tensor("kt_gat", [2, 128, 2 * LQ], BF16)
        v_bounce = nc.dram_tensor("v_bounce", [128, vcols], BF16)
        v_gat = nc.dram_tensor("v_gat", [2, 128, vcols], BF16)

    with tile.TileContext(nc) as tc, ExitStack() as ctx:
        const = ctx.enter_context(tc.tile_pool(name="const", bufs=1))
        xt_pool = ctx.enter_context(tc.tile_pool(name="xt", bufs=1))
        psum = ctx.enter_context(tc.tile_pool(name="psum", bufs=4, space="PSUM"))
        opsum = ctx.enter_context(tc.tile_pool(name="opsum", bufs=4, space="PSUM"))
        ptile_pool = ctx.enter_context(tc.tile_pool(name="ptile", bufs=6))
        small = ctx.enter_context(tc.tile_pool(name="small", bufs=4))

        rg = [[0, 1], [2, 3], [4, 5], [6, 7]]

        # ---- input DMAs ---------------------------------------------------
        # All inputs ride the sync queue as full-row transfers (2KB/partition
        # descriptor runs -- column-split loads halve DMA efficiency), in
        # K-proj consumption order.  Only the K/V exchange bounce DMAs use
        # the act queue, so the AllGather triggers are never stuck behind
        # the input stream.
        wk = const.tile([128, NT, DK], BF16)
        wv = const.tile([128, NT, DV], BF16)
        wq = const.tile([128, NT, DK], BF16)
        xt_sb = xt_pool.tile([128, NT, xcols], BF16, name="xt_sb")
        xt = [xt_sb[:, dt, :] for dt in range(NT)]

        def load_xt(g0, gn):
            nc.sync.dma_start(
                xt_sb[:, g0:g0 + gn, :], xt_ext[:, g0:g0 + gn, :]
            )

        nc.sync.dma_start(wk[:, 0:4, :], wk_ext[:, 0:4, :])
        load_xt(0, 2)
        nc.sync.dma_start(wk[:, 4:NT, :], wk_ext[:, 4:NT, :])
        load_xt(2, 2)
        load_xt(4, 4)
        nc.sync.dma_start(wv[:], wv_ext[:, :, :])
        load_xt(8, 4)
        nc.sync.dma_start(wq[:], wq_ext[:, :, :])
        load_xt(12, 4)
        trimask = const.tile([128, 2, 128], F32)
        nc.sync.dma_start(trimask[:], trimask_ext.ap())

        # ---- K^T projection (local stripe): [128, 2(m), LQ] bf16 ----------
        kt_loc = const.tile([128, 2, LQ], BF16)
        kt_rem = const.tile([128, 2, LQ], BF16)
        kt_all = [kt_loc, kt_rem]

        if mode == "rdma":
            # Pair-peer exchange via direct SBUF->SBUF remote DMA pushes.
            # Receiver readiness is proven by a tiny ncfw barrier AFTER both
            # sends complete locally (lsem); a local bsem inc after the
            # barrier gives the PE a sim-modelable wait condition.
            rsem_k = nc.alloc_semaphore("rsem_k")
            rsem_v = nc.alloc_semaphore("rsem_v")
            lsem = nc.alloc_semaphore("lsem_rdma")
            bsem = nc.alloc_semaphore("bsem_rdma")
            RDESTS = [(0, 1)] + [None] * 7  # relative: same chip, tpb ^= 1
            bar_in = nc.dram_tensor("bar_in", [128, 2], BF16)
            bar_out = nc.dram_tensor("bar_out", [2, 128, 2], BF16)

        def project_kt_chain(dst, seg, m, n):
            col0 = seg * LQ + n * CHUNK
            ps = psum.tile([128, CHUNK], F32, name="ps", tag="s")
            for dt in range(NT):
                nc.tensor.matmul(
                    ps[:],
                    wk[:, dt, m * 128:(m + 1) * 128],
                    xt[dt][:, col0:col0 + CHUNK],
                    start=(dt == 0),
                    stop=(dt == NT - 1),
                )
            nc.scalar.copy(dst[:, m, n * CHUNK:(n + 1) * CHUNK], ps[:])

        def project_kt(dst, seg):
            for n in range(LQ // CHUNK):
                for m in range(2):
                    project_kt_chain(dst, seg, m, n)

        # ---- V projection (local stripe): [128, KSEG, 257] bf16 ------------
        v_loc = const.tile([128, KSEG, DV + 1], BF16)
        v_rem = const.tile([128, KSEG, DV + 1], BF16)
        v_all = [v_loc, v_rem]

        def project_v_chain(dst, seg, rt):
            col0 = seg * LQ + rt * 128
            ps = psum.tile([128, DV], F32, name="ps", tag="s")
            for dt in range(NT):
                nc.tensor.matmul(
                    ps[:],
                    xt[dt][:, col0:col0 + 128],
                    wv[:, dt, :],
                    start=(dt == 0),
                    stop=(dt == NT - 1),
                )
            nc.scalar.copy(dst[:, rt, 0:DV], ps[:])

        def project_v(dst, seg):
            nc.vector.memset(dst[:, :, DV:DV + 1], 1.0)
            for rt in range(KSEG):
                project_v_chain(dst, seg, rt)

        project_kt(kt_loc, 0)
        if mode == "coll2":
            nc.scalar.dma_start(kt_bounce[:, :], kt_loc.rearrange("p m q -> p (m q)"))
            nc.gpsimd.collective_compute(
                "AllGather", mybir.AluOpType.bypass, replica_groups=rg,
                ins=[kt_bounce.ap()], outs=[kt_gat.ap()],
            )
        elif mode == "rdma":
            nc.gpsimd.remote_dma_broadcast(
                kt_rem.rearrange("p m q -> p (m q)"),
                kt_loc.rearrange("p m q -> p (m q)"),
                remote_sem=rsem_k, local_sem=lsem, rdests=RDESTS,
            )
            nc.gpsimd.trigger_dma(count=None)
        project_v(v_loc, 0)
        if mode == "coll2":
            nc.scalar.dma_start(
                v_bounce[:, :], v_loc.rearrange("p t c -> p (t c)")
            )
            nc.gpsimd.collective_compute(
                "AllGather", mybir.AluOpType.bypass, replica_groups=rg,
                ins=[v_bounce.ap()], outs=[v_gat.ap()],
            )
        elif mode == "rdma":
            nc.gpsimd.remote_dma_broadcast(
                v_rem.rearrange("p t c -> p (t c)"),
                v_loc.rearrange("p t c -> p (t c)"),
                remote_sem=rsem_v, local_sem=lsem, rdests=RDESTS,
            )
            nc.gpsimd.trigger_dma(count=None)
            # barrier with the peer: its completion (ncfw floor ~5us plus the
            # ~45us init wall, >>10us after the push triggers) proves the
            # peer's pushes have landed here
            nc.gpsimd.collective_compute(
                "AllGather", mybir.AluOpType.bypass, replica_groups=rg,
                ins=[bar_in.ap()], outs=[bar_out.ap()],
            )
            nc.gpsimd.sem_inc(bsem, 1)

        # ---- Q^T projection: [128, 2(m), LQ] bf16 --------------------------
        qt = const.tile([128, 2, LQ], BF16)
        for n in range(LQ // CHUNK):
            for m in range(2):
                ps = psum.tile([128, CHUNK], F32, name="ps", tag="s")
                for dt in range(NT):
                    nc.tensor.matmul(
                        ps[:],
                        wq[:, dt, m * 128:(m + 1) * 128],
                        xt[dt][:, n * CHUNK:n * CHUNK + CHUNK],
                        start=(dt == 0),
                        stop=(dt == NT - 1),
                    )
                nc.scalar.copy(qt[:, m, n * CHUNK:(n + 1) * CHUNK], ps[:])

        # ---- fetch the pair peer's gathered K^T and V directly ------------
        # Two conditional DMAs per tensor: per-core CONDS input decides which
        # AllGather slot is the peer's; the other DMA is skipped entirely
        # (cond -> skip_entire_dma; the skipped DMA still bumps semaphores,
        # keeping Tile's dependency bookkeeping intact).  This replaces the
        # old readback-both-slots + 3-op DVE select per tensor.
        if mode == "coll2":
            cond_regs = []
            for i in range(2):
                r = nc.sync.alloc_register(f"peer_cond_{i}")
                nc.sync.reg_load(r, conds_ext[0:1, i:i + 1])
                cond_regs.append(
                    nc.sync.snap(r, donate=True, min_val=0, max_val=1)
                )
            kt_rem_flat = kt_rem.rearrange("p m q -> p (m q)")
            v_rem_flat = v_rem.rearrange("p t c -> p (t c)")
            nc.sync.dma_start(kt_rem_flat[:], kt_gat[0, :, :], cond=cond_regs[0])
            nc.sync.dma_start(kt_rem_flat[:], kt_gat[1, :, :], cond=cond_regs[1])
            nc.sync.dma_start(v_rem_flat[:], v_gat[0, :, :], cond=cond_regs[0])
            nc.sync.dma_start(v_rem_flat[:], v_gat[1, :, :], cond=cond_regs[1])
        elif mode == "dup":
            project_kt(kt_rem, 1)
            project_v(v_rem, 1)

        # ---- attention -----------------------------------------------------
        # S^T tiles [128 keys, <=512 queries]; P^T = exp(S^T/16 [+ trimask]);
        # O_aug[q] [128 q, 257] accumulates P^T.T @ V_aug over (seg, kb).
        # Diagonal-band tiles (j = kb - 4c >= 0) only compute the visible
        # query range [128j:512]; their first 128-col block gets the additive
        # triangle mask.
        def scores_exp(c, seg, kb, p_out):
            j = kb - Q4 * c
            if j > 0:
                cols = CHUNK - j * 128
                q0 = c * CHUNK + j * 128
                o0 = j * 128
            else:
                cols = CHUNK
                q0 = c * CHUNK
                o0 = 0
            s_ps = psum.tile([128, cols], F32, name="ps", tag="s")
            for m in range(2):
                nc.tensor.matmul(
                    s_ps[:],
                    kt_all[seg][:, m, kb * 128:(kb + 1) * 128],
                    qt[:, m, q0:q0 + cols],
                    start=(m == 0),
                    stop=(m == 1),
                )
            if j >= 0:
                # diagonal block = first 128 cols of the (trimmed) range
                nc.vector.tensor_add(
                    s_ps[:, 0:128], s_ps[:, 0:128], trimask[:, seg, :]
                )
            nc.scalar.activation(
                p_out[:, o0:o0 + cols], s_ps[:],
                mybir.ActivationFunctionType.Exp, scale=SCALE,
            )

        def av(c, seg, kb, p, o_ps):
            for q in range(Q4):
                ti = Q4 * c + q
                if kb > ti:
                    continue
                nc.tensor.matmul(
                    o_ps[q][:],
                    p[:, q * 128:(q + 1) * 128],
                    v_all[seg][:, kb, :],
                    start=(kb == 0),
                    stop=(kb == ti),
                )

        def normalize_q(c, q, o_ps, stash):
            # o = (o_seg1 + stashed_seg0); out = o[:, :DV] / o[:, DV]
            nc.vector.tensor_add(o_ps[q][:], o_ps[q][:], stash[q][:])
            recip = small.tile([128, 1], F32, name="recip")
            nc.vector.reciprocal(recip[:], o_ps[q][:, DV:DV + 1])
            o_sb = small.tile([128, DV], F32, name="o_sb")
            nc.vector.tensor_scalar_mul(o_sb[:], o_ps[q][:, 0:DV], recip[:])
            r0 = (Q4 * c + q) * 128
            nc.sync.dma_start(out_ext[r0:r0 + 128, :], o_sb[:])

        # Local-segment attention, software-pipelined so the PE is never
        # waiting on the exp of the tile it is about to consume: scores run
        # LOOK tiles ahead of the AV accumulations.
        o_stash = {}
        LOOK = 2

        def seg0_chunk(c):
            o_ps = [
                opsum.tile([128, DV + 1], F32, name="o_ps", tag="o")
                for _ in range(Q4)
            ]
            kbmax = Q4 * (c + 1)
            plist = {}

            def do_av(kb):
                av(c, 0, kb, plist[kb], o_ps)
                for q in range(Q4):
                    if kb == Q4 * c + q:  # accumulation for q just stopped
                        st = const.tile(
                            [128, DV + 1], F32, name=f"o_stash_{c}_{q}"
                        )
                        o_stash[(c, q)] = st
                        nc.vector.tensor_copy(st[:], o_ps[q][:])

            for kb in range(kbmax):
                p = ptile_pool.tile([128, CHUNK], BF16, name="p")
                plist[kb] = p
                scores_exp(c, 0, kb, p)
                if kb >= LOOK:
                    do_av(kb - LOOK)
            for kb in range(max(0, kbmax - LOOK), kbmax):
                do_av(kb)

        for c in range(NCHUNK):
            seg0_chunk(c)

        # Remote segment: all scores first (they only need K^T_rem, which
        # arrives well before V_rem), then the AV accumulations + normalize.
        if mode == "rdma":
            nc.tensor.wait_ge(bsem, 1)
        p_store = {}
        for c in range(NCHUNK):
            for kb in range(Q4 * (c + 1)):
                t = const.tile([128, CHUNK], BF16, name=f"p_rem_{c}_{kb}")
                p_store[(c, kb)] = t
                scores_exp(c, 1, kb, t)
        for c in range(NCHUNK):
            o_ps = [
                opsum.tile([128, DV + 1], F32, name="o_ps", tag="o")
                for _ in range(Q4)
            ]
            for kb in range(Q4 * (c + 1)):
                av(c, 1, kb, p_store[(c, kb)], o_ps)
                if kb >= Q4 * c:
                    normalize_q(c, kb - Q4 * c, o_ps, [o_stash[(c, q)] for q in range(Q4)])

    nc.finalize()
    return nc


_CACHED = {}


def _get_kernel(mode: str):
    if mode not in _CACHED:
        _CACHED[mode] = build_kernel(mode)
    return _CACHED[mode]


def _prepare_in_maps(X, W_Q, W_K, W_V, mode):
    def wlayout(W):
        # w[p, dt, c] = W[dt*128 + p, c]
        n = W.shape[1]
        return np.ascontiguousarray(
            W.reshape(NT, 128, n).transpose(1, 0, 2)
        ).astype(ml_dtypes.bfloat16)

    wq = wlayout(W_Q)
    wk = wlayout(W_K)
    wv = wlayout(W_V)

    # Triangle masks for the diagonal 128x128 block of diagonal-band tiles.
    # Element [p, x]: key-in-block p, query-in-block x.
    # seg0 (own stripe): visible iff p <= x.
    # seg1 (remote stripe): s=0 cores: visible iff p < x; s=1: p <= x.
    p_idx = np.arange(128)[:, None]
    x_idx = np.arange(128)[None, :]
    tri_incl = np.where(p_idx <= x_idx, 0.0, MASK).astype(np.float32)
    tri_excl = np.where(p_idx < x_idx, 0.0, MASK).astype(np.float32)

    in_maps = []
    for core in range(8):
        b, s = core // 2, core % 2
        # partition-major layout: xt[p, dt, r] = X[b, stripe r, dt*128 + p]
        loc = X[b, s::2, :].reshape(LQ, NT, 128).transpose(2, 1, 0)
        if mode == "coll2":
            xt = np.ascontiguousarray(loc).astype(ml_dtypes.bfloat16)
        else:
            remo = X[b, 1 - s::2, :].reshape(LQ, NT, 128).transpose(2, 1, 0)
            xt = np.concatenate([loc, remo], axis=2).astype(ml_dtypes.bfloat16)
        trimask = np.stack(
            [tri_incl, tri_excl if s == 0 else tri_incl], axis=1
        )  # [128, 2, 128]
        conds = np.zeros((1, 2), np.uint32)
        conds[0, 1 - s] = 1  # the pair peer's slot in the gather
        in_maps.append(
            {"XT": xt, "WQ": wq, "WK": wk, "WV": wv,
             "TRIMASK": np.ascontiguousarray(trimask), "CONDS": conds}
        )
    return in_maps


def _assemble(results):
    Z = np.empty((B, L, DV), np.float32)
    for core in range(8):
        b, s = core // 2, core % 2
        Z[b, s::2, :] = results[core]["OUT"]
    return Z


def kernel(X, W_Q, W_K, W_V):
    nc = _get_kernel(MODE)
    in_maps = _prepare_in_maps(X, W_Q, W_K, W_V, MODE)
    res = run_bass_kernel_spmd(nc, in_maps, core_ids=list(range(8)))
    return _assemble(res.results)
